# revision 14
# baseline (speedup 1.0000x reference)
"""Expert-parallel Switch-Transformer MoE layer for 8 Trainium2 NeuronCores.

Strategy: every core redundantly computes the (cheap) router for all tokens in
exact fp32, then handles exactly one expert: it compacts its token list with a
gpsimd sparse_gather (order-preserving -> exact first-come capacity semantics),
gathers + pmax-scales the token rows (ReLU positive homogeneity lets the
combine scale fold into the FFN input), runs the two expert matmuls in f32r
(TF32), and returns the expert output in transposed layout plus the slot->token
map. The host scatters each expert's rows over a copy of x (dropped tokens keep
the residual).
"""
import sys

sys.path.insert(0, "/opt/trn_rl_repo")

import numpy as np

import concourse.bass as bass
import concourse.bacc as bacc
import concourse.mybir as mybir
import concourse.tile as tile
from concourse.bass import IndirectOffsetOnAxis
from concourse.bass_utils import run_bass_kernel_spmd
from concourse.masks import make_identity

F32 = mybir.dt.float32
F32R = mybir.dt.float32r
I32 = mybir.dt.int32
U32 = mybir.dt.uint32
AF = mybir.ActivationFunctionType
ALU = mybir.AluOpType
AX = mybir.AxisListType

B, S, D, FF, E = 4, 2048, 1024, 4096, 8
T = B * S                       # 8192 tokens
CAP = int(1.25 * T / E)         # 1280 capacity/expert
NT = T // 128                   # 64 token tiles
NG = CAP // 128                 # 10 slot tiles
NCH = [(0, 512), (512, 512), (1024, 256)]   # cap chunks (moving-dim <= 512)
KD = D // 128                   # 8 contraction tiles over d_model
NFG = 4                         # f-groups of 1024
FM = 8                          # f tiles per group

_CACHE = {}


def _build():
    nc = bacc.Bacc("TRN2", target_bir_lowering=False)

    XT = nc.dram_tensor("xT", [D, T], F32, kind="ExternalInput")
    X2 = nc.dram_tensor("x2", [T, D], F32, kind="ExternalInput")
    WR = nc.dram_tensor("wr", [D, E], F32, kind="ExternalInput")
    W1 = nc.dram_tensor("w1", [D, FF], F32R, kind="ExternalInput")
    W2 = nc.dram_tensor("w2", [FF, D], F32R, kind="ExternalInput")
    EOH = nc.dram_tensor("eoh", [128, 512], F32, kind="ExternalInput")
    IOTF = nc.dram_tensor("iotf", [128, NT], F32, kind="ExternalInput")
    UT = nc.dram_tensor("ut", [128, 128], F32, kind="ExternalInput")
    ONES1 = nc.dram_tensor("ones1", [1, 128], F32, kind="ExternalInput")

    EOUTT = nc.dram_tensor("eoutT", [D, CAP], F32, kind="ExternalOutput")
    TOK = nc.dram_tensor("tok", [128, NG], I32, kind="ExternalOutput")
    CNT = nc.dram_tensor("cnt", [1, 1], U32, kind="ExternalOutput")

    TOKM2 = nc.dram_tensor("tokm2", [CAP, 64], F32)

    with tile.TileContext(nc) as tc:
        with tc.tile_pool(name="persist", bufs=1) as pp:
            ident = pp.tile([128, 128], F32)
            make_identity(nc, ident[:])
            dispT = pp.tile([128, KD * CAP], F32R)     # [d-part, k*CAP+slot]
            h16 = pp.tile([128, FM * CAP], F32R)       # [f-part, fm*CAP+slot]
            eacc = pp.tile([128, KD * CAP], F32)       # [d-part, dm*CAP+slot]

            # ---------------- Phase R: router (exact fp32) ----------------
            with tc.tile_pool(name="rt", bufs=1) as rp, \
                 tc.tile_pool(name="xtp", bufs=2) as xp, \
                 tc.tile_pool(name="rps", bufs=3, space="PSUM") as rpsum:
                wr_sb = rp.tile([128, KD * E], F32)
                for k in range(KD):
                    nc.sync.dma_start(wr_sb[:, k * E:(k + 1) * E],
                                      WR[k * 128:(k + 1) * 128, :])
                eoh_sb = rp.tile([128, 512], F32)
                nc.sync.dma_start(eoh_sb[:], EOH[:])
                iot_sb = pp.tile([128, NT], F32)
                nc.sync.dma_start(iot_sb[:], IOTF[:])
                ut_sb = pp.tile([128, 128], F32)
                nc.sync.dma_start(ut_sb[:], UT[:])
                ones1_sb = pp.tile([1, 128], F32)
                nc.sync.dma_start(ones1_sb[:], ONES1[:])

                lacc = rp.tile([128, NT * E], F32)     # fp32 logits, SBUF
                for k in range(KD):
                    xt_k = xp.tile([128, T], F32)
                    nc.sync.dma_start(xt_k[:], XT[k * 128:(k + 1) * 128, :])
                    lg = rpsum.tile([128, NT * E], F32, tag="lg")
                    for m in range(NT):
                        nc.tensor.matmul(
                            lg[:, m * E:(m + 1) * E],
                            xt_k[:, m * 128:(m + 1) * 128],
                            wr_sb[:, k * E:(k + 1) * E],
                            start=True, stop=True)
                    if k == 0:
                        nc.vector.tensor_copy(lacc[:], lg[:])
                    else:
                        nc.vector.tensor_add(lacc[:], lacc[:], lg[:])

                expt = rp.tile([128, NT * E], F32)
                nc.scalar.activation(expt[:], lacc[:], AF.Exp)
                e3 = expt[:].rearrange("p (i e) -> p i e", e=E)
                sumex = rp.tile([128, NT], F32)
                nc.vector.tensor_reduce(sumex[:], e3, axis=AX.X, op=ALU.add)
                maxex = rp.tile([128, NT], F32)
                nc.vector.tensor_reduce(maxex[:], e3, axis=AX.X, op=ALU.max)
                lemul = rp.tile([128, NT * E], F32)
                nc.vector.tensor_tensor(out=lemul[:], in0=expt[:],
                                        in1=eoh_sb[:], op=ALU.mult)
                le = rp.tile([128, NT], F32)
                nc.vector.tensor_reduce(
                    le[:], lemul[:].rearrange("p (i e) -> p i e", e=E),
                    axis=AX.X, op=ALU.add)
                eq = pp.tile([128, NT], F32)
                nc.vector.tensor_tensor(out=eq[:], in0=le[:], in1=maxex[:],
                                        op=ALU.is_equal)
                rcp = rp.tile([128, NT], F32)
                nc.vector.reciprocal(rcp[:], sumex[:])
                pmax = pp.tile([128, NT], F32)
                nc.vector.tensor_tensor(out=pmax[:], in0=maxex[:], in1=rcp[:],
                                        op=ALU.mult)

            # ------------- Phase D: slot assignment + compaction -------------
            # slot(t) = (# earlier tokens of this expert) computed exactly as
            # an inclusive prefix sum: within-tile via a triangular matmul,
            # across tiles via a 64-wide scan; then (token_id, pmax) rows are
            # indirect-DMA-scattered to their slot (row padded to 256 B so the
            # scatter is not descriptor-bound). Non-members and over-capacity
            # tokens get slot >= CAP and are dropped by the bounds check.
            with tc.tile_pool(name="dph", bufs=1) as dp, \
                 tc.tile_pool(name="dpp", bufs=3) as dpp, \
                 tc.tile_pool(name="dps", bufs=2, space="PSUM") as dps:
                inclp = dps.tile([128, NT], F32, tag="inclp")
                nc.tensor.matmul(inclp[:], ut_sb[:], eq[:], start=True,
                                 stop=True)
                incl = dp.tile([128, NT], F32)
                nc.vector.tensor_copy(incl[:], inclp[:])
                totr = dp.tile([1, NT], F32)
                nc.sync.dma_start(totr[:], incl[127:128, :])
                zrow = dp.tile([1, NT], F32)
                nc.vector.memset(zrow[:], 0.0)
                scanr = dp.tile([1, NT], F32)
                nc.vector.tensor_tensor_scan(
                    scanr[:], totr[:], zrow[:], 0.0,
                    op0=ALU.add, op1=ALU.add)
                exclr = dp.tile([1, NT], F32)
                nc.vector.tensor_tensor(out=exclr[:], in0=scanr[:],
                                        in1=totr[:], op=ALU.subtract)
                cntu = dp.tile([1, 1], U32)
                nc.vector.tensor_copy(cntu[:], scanr[:, NT - 1:NT])
                nc.sync.dma_start(CNT[:], cntu[:])
                bcast = dps.tile([128, NT], F32, tag="bcast")
                nc.tensor.matmul(bcast[:], ones1_sb[:], exclr[:], start=True,
                                 stop=True)
                slotp1 = dp.tile([128, NT], F32)
                nc.vector.tensor_add(slotp1[:], incl[:], bcast[:])
                scat = dp.tile([128, NT], F32)
                nc.vector.tensor_scalar_add(scat[:], slotp1[:],
                                            float(-(CAP + 1)))
                nc.vector.tensor_tensor(out=scat[:], in0=eq[:], in1=scat[:],
                                        op=ALU.mult)
                nc.vector.tensor_scalar_add(scat[:], scat[:], float(CAP))
                scati = dp.tile([128, NT], I32)
                nc.vector.tensor_copy(scati[:], scat[:])
                for i in range(NT):
                    pt = dpp.tile([128, 64], F32, tag="pt")
                    nc.vector.memset(pt[:, 2:64], 0.0)
                    nc.vector.tensor_copy(pt[:, 0:1], iot_sb[:, i:i + 1])
                    nc.vector.tensor_copy(pt[:, 1:2], pmax[:, i:i + 1])
                    nc.gpsimd.indirect_dma_start(
                        out=TOKM2[:], out_offset=IndirectOffsetOnAxis(
                            ap=scati[:, i:i + 1], axis=0),
                        in_=pt[:], in_offset=None,
                        bounds_check=CAP - 1, oob_is_err=False)
                tokf = dp.tile([128, NG], F32)
                nc.sync.dma_start(
                    tokf[:], TOKM2[:, 0:1]
                    .rearrange("(g p) one -> p (g one)", p=128))
                pmxs = dp.tile([128, NG], F32)
                nc.sync.dma_start(
                    pmxs[:], TOKM2[:, 1:2]
                    .rearrange("(g p) one -> p (g one)", p=128))
                toki_raw = dp.tile([128, NG], I32)
                nc.vector.tensor_copy(toki_raw[:], tokf[:])
                nc.sync.dma_start(TOK[:], toki_raw[:])
                tokcl = dp.tile([128, NG], F32)
                nc.vector.tensor_scalar(out=tokcl[:], in0=tokf[:],
                                        scalar1=0.0, scalar2=float(T - 1),
                                        op0=ALU.max, op1=ALU.min)
                toki = dp.tile([128, NG], I32)
                nc.vector.tensor_copy(toki[:], tokcl[:])

                # ---------- Phase G: gather rows, scale, transpose ----------
                with tc.tile_pool(name="gth", bufs=3) as gp, \
                     tc.tile_pool(name="gps", bufs=2, space="PSUM") as gpsm:
                    for g in range(NG):
                        dsp = gp.tile([128, D], F32)
                        nc.gpsimd.indirect_dma_start(
                            out=dsp[:], out_offset=None, in_=X2[:],
                            in_offset=IndirectOffsetOnAxis(
                                ap=toki[:, g:g + 1], axis=0))
                        nc.vector.tensor_scalar_mul(dsp[:], dsp[:],
                                                    pmxs[:, g:g + 1])
                        for k in range(KD):
                            tp = gpsm.tile([128, 128], F32)
                            nc.tensor.transpose(
                                tp[:], dsp[:, k * 128:(k + 1) * 128], ident[:])
                            nc.vector.tensor_copy(
                                dispT[:, k * CAP + g * 128:
                                      k * CAP + (g + 1) * 128], tp[:])

            # ---------------- Phase F: expert FFN (f32r) ----------------
            with tc.tile_pool(name="w1p", bufs=9) as w1p, \
                 tc.tile_pool(name="w2p", bufs=9) as w2p, \
                 tc.tile_pool(name="fps", bufs=3, space="PSUM") as fps:
                for fg in range(NFG):
                    w1g = []
                    for k in range(KD):
                        t = w1p.tile([128, 1024], F32R, tag="w1g")
                        nc.sync.dma_start(
                            t[:], W1[k * 128:(k + 1) * 128,
                                     fg * 1024:(fg + 1) * 1024])
                        w1g.append(t)
                    w2g = []
                    for k in range(FM):
                        t = w2p.tile([128, 1024], F32R, tag="w2g")
                        nc.sync.dma_start(
                            t[:], W2[fg * 1024 + k * 128:
                                     fg * 1024 + (k + 1) * 128, :])
                        w2g.append(t)
                    for fm in range(FM):
                        for noff, ncnt in NCH:
                            hp = fps.tile([128, 512], F32, tag="hp")
                            for k in range(KD):
                                nc.tensor.matmul(
                                    hp[:, :ncnt],
                                    w1g[k][:, fm * 128:(fm + 1) * 128],
                                    dispT[:, k * CAP + noff:
                                          k * CAP + noff + ncnt],
                                    start=(k == 0), stop=(k == KD - 1))
                            nc.scalar.activation(
                                h16[:, fm * CAP + noff:fm * CAP + noff + ncnt],
                                hp[:, :ncnt], AF.Relu)
                    for dm in range(KD):
                        for noff, ncnt in NCH:
                            ep = fps.tile([128, 512], F32, tag="ep")
                            for fk in range(FM):
                                nc.tensor.matmul(
                                    ep[:, :ncnt],
                                    w2g[fk][:, dm * 128:(dm + 1) * 128],
                                    h16[:, fk * CAP + noff:
                                        fk * CAP + noff + ncnt],
                                    start=(fk == 0), stop=(fk == FM - 1))
                            dst = eacc[:, dm * CAP + noff:dm * CAP + noff + ncnt]
                            if fg == 0:
                                nc.vector.tensor_copy(dst, ep[:, :ncnt])
                            else:
                                nc.vector.tensor_add(dst, dst, ep[:, :ncnt])
                for dm in range(KD):
                    nc.sync.dma_start(EOUTT[dm * 128:(dm + 1) * 128, :],
                                      eacc[:, dm * CAP:(dm + 1) * CAP])

    nc.compile()
    return nc


def get_nc():
    if "nc" not in _CACHE:
        _CACHE["nc"] = _build()
    return _CACHE["nc"]


def kernel(x, Wr, W1, W2):
    nc = get_nc()
    x2 = np.ascontiguousarray(np.asarray(x, np.float32).reshape(T, D))
    xT = np.ascontiguousarray(x2.T)
    Wr = np.asarray(Wr, np.float32)
    W1 = np.asarray(W1, np.float32)
    W2 = np.asarray(W2, np.float32)
    iotf = (np.arange(NT, dtype=np.float32)[None, :] * 128
            + np.arange(128, dtype=np.float32)[:, None])
    ut = np.triu(np.ones((128, 128), np.float32))   # ut[k, m] = 1 iff k <= m
    ones1 = np.ones((1, 128), np.float32)
    in_maps = []
    for e in range(E):
        eoh = np.zeros((128, 512), np.float32)
        eoh[:, e::E] = 1.0
        in_maps.append({
            "xT": xT, "x2": x2, "wr": Wr,
            "w1": np.ascontiguousarray(W1[e]),
            "w2": np.ascontiguousarray(W2[e]),
            "eoh": eoh, "iotf": iotf, "ut": ut, "ones1": ones1,
        })
    res = run_bass_kernel_spmd(nc, in_maps, list(range(E))).results

    out = x2.copy()
    slots = np.arange(CAP)
    for e in range(E):
        r = res[e]
        cnt = min(int(r["cnt"][0, 0]), CAP)
        tok = np.ascontiguousarray(r["tok"].T).ravel()
        eout = np.ascontiguousarray(r["eoutT"].T)          # [CAP, D]
        valid = (slots < cnt) & (tok >= 0) & (tok < T)
        out[tok[valid]] = eout[valid]
    return out.reshape(B, S, D)


# revision 15
# speedup vs baseline: 78.1035x; 78.1035x over previous
"""Expert-parallel Switch-Transformer MoE layer for 8 Trainium2 NeuronCores.

Strategy: every core redundantly computes the (cheap) router for all tokens in
exact fp32, then handles exactly one expert: it compacts its token list with a
gpsimd sparse_gather (order-preserving -> exact first-come capacity semantics),
gathers + pmax-scales the token rows (ReLU positive homogeneity lets the
combine scale fold into the FFN input), runs the two expert matmuls in f32r
(TF32), and returns the expert output in transposed layout plus the slot->token
map. The host scatters each expert's rows over a copy of x (dropped tokens keep
the residual).
"""
import sys

sys.path.insert(0, "/opt/trn_rl_repo")

import numpy as np

import concourse.bass as bass
import concourse.bacc as bacc
import concourse.mybir as mybir
import concourse.tile as tile
from concourse.bass import IndirectOffsetOnAxis
from concourse.bass_utils import run_bass_kernel_spmd
from concourse.masks import make_identity

F32 = mybir.dt.float32
F32R = mybir.dt.float32r
I32 = mybir.dt.int32
U32 = mybir.dt.uint32
AF = mybir.ActivationFunctionType
ALU = mybir.AluOpType
AX = mybir.AxisListType

B, S, D, FF, E = 4, 2048, 1024, 4096, 8
T = B * S                       # 8192 tokens
CAP = int(1.25 * T / E)         # 1280 capacity/expert
NT = T // 128                   # 64 token tiles
NG = CAP // 128                 # 10 slot tiles
NCH = [(0, 512), (512, 512), (1024, 256)]   # cap chunks (moving-dim <= 512)
KD = D // 128                   # 8 contraction tiles over d_model
NFG = 4                         # f-groups of 1024
FM = 8                          # f tiles per group

_CACHE = {}


def _build(reps=1):
    nc = bacc.Bacc("TRN2", target_bir_lowering=False)

    XT = nc.dram_tensor("xT", [D, T], F32, kind="ExternalInput")
    X2 = nc.dram_tensor("x2", [T, D], F32, kind="ExternalInput")
    WR = nc.dram_tensor("wr", [D, E], F32, kind="ExternalInput")
    W1 = nc.dram_tensor("w1", [D, FF], F32R, kind="ExternalInput")
    W2 = nc.dram_tensor("w2", [FF, D], F32R, kind="ExternalInput")
    EOH = nc.dram_tensor("eoh", [128, 512], F32, kind="ExternalInput")
    IOTF = nc.dram_tensor("iotf", [128, NT], F32, kind="ExternalInput")
    UT = nc.dram_tensor("ut", [128, 128], F32, kind="ExternalInput")
    ONES1 = nc.dram_tensor("ones1", [1, 128], F32, kind="ExternalInput")

    EOUTT = nc.dram_tensor("eoutT", [D, CAP], F32, kind="ExternalOutput")
    TOK = nc.dram_tensor("tok", [128, NG], I32, kind="ExternalOutput")
    CNT = nc.dram_tensor("cnt", [1, 1], U32, kind="ExternalOutput")

    TOKM2 = nc.dram_tensor("tokm2", [CAP, 64], F32)

    with tile.TileContext(nc) as tc:
      for _rep in range(reps):
        with tc.tile_pool(name="persist", bufs=1) as pp:
            ident = pp.tile([128, 128], F32)
            make_identity(nc, ident[:])
            dispT = pp.tile([128, KD * CAP], F32R)     # [d-part, k*CAP+slot]
            h16 = pp.tile([128, FM * CAP], F32R)       # [f-part, fm*CAP+slot]
            eacc = pp.tile([128, KD * CAP], F32)       # [d-part, dm*CAP+slot]

            # ---------------- Phase R: router (exact fp32) ----------------
            with tc.tile_pool(name="rt", bufs=1) as rp, \
                 tc.tile_pool(name="xtp", bufs=2) as xp, \
                 tc.tile_pool(name="rps", bufs=3, space="PSUM") as rpsum:
                wr_sb = rp.tile([128, KD * E], F32)
                for k in range(KD):
                    nc.sync.dma_start(wr_sb[:, k * E:(k + 1) * E],
                                      WR[k * 128:(k + 1) * 128, :])
                eoh_sb = rp.tile([128, 512], F32)
                nc.sync.dma_start(eoh_sb[:], EOH[:])
                iot_sb = pp.tile([128, NT], F32)
                nc.sync.dma_start(iot_sb[:], IOTF[:])
                ut_sb = pp.tile([128, 128], F32)
                nc.sync.dma_start(ut_sb[:], UT[:])
                ones1_sb = pp.tile([1, 128], F32)
                nc.sync.dma_start(ones1_sb[:], ONES1[:])

                lacc = rp.tile([128, NT * E], F32)     # fp32 logits, SBUF
                for k in range(KD):
                    xt_k = xp.tile([128, T], F32)
                    nc.sync.dma_start(xt_k[:], XT[k * 128:(k + 1) * 128, :])
                    lg = rpsum.tile([128, NT * E], F32, tag="lg")
                    for m in range(NT):
                        nc.tensor.matmul(
                            lg[:, m * E:(m + 1) * E],
                            xt_k[:, m * 128:(m + 1) * 128],
                            wr_sb[:, k * E:(k + 1) * E],
                            start=True, stop=True)
                    if k == 0:
                        nc.vector.tensor_copy(lacc[:], lg[:])
                    else:
                        nc.vector.tensor_add(lacc[:], lacc[:], lg[:])

                expt = rp.tile([128, NT * E], F32)
                nc.scalar.activation(expt[:], lacc[:], AF.Exp)
                e3 = expt[:].rearrange("p (i e) -> p i e", e=E)
                sumex = rp.tile([128, NT], F32)
                nc.vector.tensor_reduce(sumex[:], e3, axis=AX.X, op=ALU.add)
                maxex = rp.tile([128, NT], F32)
                nc.vector.tensor_reduce(maxex[:], e3, axis=AX.X, op=ALU.max)
                lemul = rp.tile([128, NT * E], F32)
                nc.vector.tensor_tensor(out=lemul[:], in0=expt[:],
                                        in1=eoh_sb[:], op=ALU.mult)
                le = rp.tile([128, NT], F32)
                nc.vector.tensor_reduce(
                    le[:], lemul[:].rearrange("p (i e) -> p i e", e=E),
                    axis=AX.X, op=ALU.add)
                eq = pp.tile([128, NT], F32)
                nc.vector.tensor_tensor(out=eq[:], in0=le[:], in1=maxex[:],
                                        op=ALU.is_equal)
                rcp = rp.tile([128, NT], F32)
                nc.vector.reciprocal(rcp[:], sumex[:])
                pmax = pp.tile([128, NT], F32)
                nc.vector.tensor_tensor(out=pmax[:], in0=maxex[:], in1=rcp[:],
                                        op=ALU.mult)

            # ------------- Phase D: slot assignment + compaction -------------
            # slot(t) = (# earlier tokens of this expert) computed exactly as
            # an inclusive prefix sum: within-tile via a triangular matmul,
            # across tiles via a 64-wide scan; then (token_id, pmax) rows are
            # indirect-DMA-scattered to their slot (row padded to 256 B so the
            # scatter is not descriptor-bound). Non-members and over-capacity
            # tokens get slot >= CAP and are dropped by the bounds check.
            with tc.tile_pool(name="dph", bufs=1) as dp, \
                 tc.tile_pool(name="dpp", bufs=3) as dpp, \
                 tc.tile_pool(name="dps", bufs=2, space="PSUM") as dps:
                inclp = dps.tile([128, NT], F32, tag="inclp")
                nc.tensor.matmul(inclp[:], ut_sb[:], eq[:], start=True,
                                 stop=True)
                incl = dp.tile([128, NT], F32)
                nc.vector.tensor_copy(incl[:], inclp[:])
                totr = dp.tile([1, NT], F32)
                nc.sync.dma_start(totr[:], incl[127:128, :])
                zrow = dp.tile([1, NT], F32)
                nc.vector.memset(zrow[:], 0.0)
                scanr = dp.tile([1, NT], F32)
                nc.vector.tensor_tensor_scan(
                    scanr[:], totr[:], zrow[:], 0.0,
                    op0=ALU.add, op1=ALU.add)
                exclr = dp.tile([1, NT], F32)
                nc.vector.tensor_tensor(out=exclr[:], in0=scanr[:],
                                        in1=totr[:], op=ALU.subtract)
                cntu = dp.tile([1, 1], U32)
                nc.vector.tensor_copy(cntu[:], scanr[:, NT - 1:NT])
                nc.sync.dma_start(CNT[:], cntu[:])
                bcast = dps.tile([128, NT], F32, tag="bcast")
                nc.tensor.matmul(bcast[:], ones1_sb[:], exclr[:], start=True,
                                 stop=True)
                slotp1 = dp.tile([128, NT], F32)
                nc.vector.tensor_add(slotp1[:], incl[:], bcast[:])
                scat = dp.tile([128, NT], F32)
                nc.vector.tensor_scalar_add(scat[:], slotp1[:],
                                            float(-(CAP + 1)))
                nc.vector.tensor_tensor(out=scat[:], in0=eq[:], in1=scat[:],
                                        op=ALU.mult)
                nc.vector.tensor_scalar_add(scat[:], scat[:], float(CAP))
                scati = dp.tile([128, NT], I32)
                nc.vector.tensor_copy(scati[:], scat[:])
                for i in range(NT):
                    pt = dpp.tile([128, 64], F32, tag="pt")
                    nc.vector.memset(pt[:, 2:64], 0.0)
                    nc.vector.tensor_copy(pt[:, 0:1], iot_sb[:, i:i + 1])
                    nc.vector.tensor_copy(pt[:, 1:2], pmax[:, i:i + 1])
                    nc.gpsimd.indirect_dma_start(
                        out=TOKM2[:], out_offset=IndirectOffsetOnAxis(
                            ap=scati[:, i:i + 1], axis=0),
                        in_=pt[:], in_offset=None,
                        bounds_check=CAP - 1, oob_is_err=False)
                tokf = dp.tile([128, NG], F32)
                nc.sync.dma_start(
                    tokf[:], TOKM2[:, 0:1]
                    .rearrange("(g p) one -> p (g one)", p=128))
                pmxs = dp.tile([128, NG], F32)
                nc.sync.dma_start(
                    pmxs[:], TOKM2[:, 1:2]
                    .rearrange("(g p) one -> p (g one)", p=128))
                toki_raw = dp.tile([128, NG], I32)
                nc.vector.tensor_copy(toki_raw[:], tokf[:])
                nc.sync.dma_start(TOK[:], toki_raw[:])
                tokcl = dp.tile([128, NG], F32)
                nc.vector.tensor_scalar(out=tokcl[:], in0=tokf[:],
                                        scalar1=0.0, scalar2=float(T - 1),
                                        op0=ALU.max, op1=ALU.min)
                toki = dp.tile([128, NG], I32)
                nc.vector.tensor_copy(toki[:], tokcl[:])

                # ---------- Phase G: gather rows, scale, transpose ----------
                with tc.tile_pool(name="gth", bufs=3) as gp, \
                     tc.tile_pool(name="gps", bufs=2, space="PSUM") as gpsm:
                    for g in range(NG):
                        dsp = gp.tile([128, D], F32)
                        nc.gpsimd.indirect_dma_start(
                            out=dsp[:], out_offset=None, in_=X2[:],
                            in_offset=IndirectOffsetOnAxis(
                                ap=toki[:, g:g + 1], axis=0))
                        nc.vector.tensor_scalar_mul(dsp[:], dsp[:],
                                                    pmxs[:, g:g + 1])
                        for k in range(KD):
                            tp = gpsm.tile([128, 128], F32)
                            nc.tensor.transpose(
                                tp[:], dsp[:, k * 128:(k + 1) * 128], ident[:])
                            nc.vector.tensor_copy(
                                dispT[:, k * CAP + g * 128:
                                      k * CAP + (g + 1) * 128], tp[:])

            # ---------------- Phase F: expert FFN (f32r) ----------------
            with tc.tile_pool(name="w1p", bufs=9) as w1p, \
                 tc.tile_pool(name="w2p", bufs=9) as w2p, \
                 tc.tile_pool(name="fps", bufs=3, space="PSUM") as fps:
                for fg in range(NFG):
                    w1g = []
                    for k in range(KD):
                        t = w1p.tile([128, 1024], F32R, tag="w1g")
                        nc.sync.dma_start(
                            t[:], W1[k * 128:(k + 1) * 128,
                                     fg * 1024:(fg + 1) * 1024])
                        w1g.append(t)
                    w2g = []
                    for k in range(FM):
                        t = w2p.tile([128, 1024], F32R, tag="w2g")
                        nc.sync.dma_start(
                            t[:], W2[fg * 1024 + k * 128:
                                     fg * 1024 + (k + 1) * 128, :])
                        w2g.append(t)
                    for fm in range(FM):
                        for noff, ncnt in NCH:
                            hp = fps.tile([128, 512], F32, tag="hp")
                            for k in range(KD):
                                nc.tensor.matmul(
                                    hp[:, :ncnt],
                                    w1g[k][:, fm * 128:(fm + 1) * 128],
                                    dispT[:, k * CAP + noff:
                                          k * CAP + noff + ncnt],
                                    start=(k == 0), stop=(k == KD - 1))
                            nc.scalar.activation(
                                h16[:, fm * CAP + noff:fm * CAP + noff + ncnt],
                                hp[:, :ncnt], AF.Relu)
                    for dm in range(KD):
                        for noff, ncnt in NCH:
                            ep = fps.tile([128, 512], F32, tag="ep")
                            for fk in range(FM):
                                nc.tensor.matmul(
                                    ep[:, :ncnt],
                                    w2g[fk][:, dm * 128:(dm + 1) * 128],
                                    h16[:, fk * CAP + noff:
                                        fk * CAP + noff + ncnt],
                                    start=(fk == 0), stop=(fk == FM - 1))
                            dst = eacc[:, dm * CAP + noff:dm * CAP + noff + ncnt]
                            if fg == 0:
                                nc.vector.tensor_copy(dst, ep[:, :ncnt])
                            else:
                                nc.vector.tensor_add(dst, dst, ep[:, :ncnt])
                for dm in range(KD):
                    nc.sync.dma_start(EOUTT[dm * 128:(dm + 1) * 128, :],
                                      eacc[:, dm * CAP:(dm + 1) * CAP])

    nc.compile()
    return nc


def get_nc(reps=1):
    if reps not in _CACHE:
        _CACHE[reps] = _build(reps)
    return _CACHE[reps]


def kernel(x, Wr, W1, W2):
    nc = get_nc()
    x2 = np.ascontiguousarray(np.asarray(x, np.float32).reshape(T, D))
    xT = np.ascontiguousarray(x2.T)
    Wr = np.asarray(Wr, np.float32)
    W1 = np.asarray(W1, np.float32)
    W2 = np.asarray(W2, np.float32)
    iotf = (np.arange(NT, dtype=np.float32)[None, :] * 128
            + np.arange(128, dtype=np.float32)[:, None])
    ut = np.triu(np.ones((128, 128), np.float32))   # ut[k, m] = 1 iff k <= m
    ones1 = np.ones((1, 128), np.float32)
    in_maps = []
    for e in range(E):
        eoh = np.zeros((128, 512), np.float32)
        eoh[:, e::E] = 1.0
        in_maps.append({
            "xT": xT, "x2": x2, "wr": Wr,
            "w1": np.ascontiguousarray(W1[e]),
            "w2": np.ascontiguousarray(W2[e]),
            "eoh": eoh, "iotf": iotf, "ut": ut, "ones1": ones1,
        })
    res = run_bass_kernel_spmd(nc, in_maps, list(range(E))).results

    out = x2.copy()
    slots = np.arange(CAP)
    for e in range(E):
        r = res[e]
        cnt = min(int(r["cnt"][0, 0]), CAP)
        tok = np.ascontiguousarray(r["tok"].T).ravel()
        eout = np.ascontiguousarray(r["eoutT"].T)          # [CAP, D]
        valid = (slots < cnt) & (tok >= 0) & (tok < T)
        out[tok[valid]] = eout[valid]
    return out.reshape(B, S, D)
